# revision 1
# baseline (speedup 1.0000x reference)
import numpy as np
import jax
import jax.numpy as jnp
from functools import partial

# Hardcoded problem dims (nn_DecoderStructural)
VOCAB = 32
EMB = 256
ENC = 512
HID = 512
ATT = 512
START = 0
B, P, T = 64, 256, 100
NCORES = 8
BS = B // NCORES  # batch shard per core


def _decode_shard(efm, tgt, embedding, We, be, Wh, bh, v_att,
                  W_ih, W_hh, b_ih, b_hh, W_fc, b_fc):
    # efm: [bs, P, ENC], tgt: [bs, T]
    enc_proj = efm @ We + be  # [bs,P,A]

    def step(carry, tgt_t):
        h, tok = carry
        att = jnp.tanh(enc_proj + (h @ Wh + bh)[:, None, :])
        scores = att @ v_att
        alpha = jax.nn.softmax(scores, axis=1)
        ctx = jnp.einsum('bp,bpe->be', alpha, efm)
        emb = embedding[tok]
        x = jnp.concatenate([ctx, emb], axis=1)
        gi = x @ W_ih.T + b_ih
        gh = h @ W_hh.T + b_hh
        ir, iz, i_n = jnp.split(gi, 3, axis=1)
        hr, hz, h_n = jnp.split(gh, 3, axis=1)
        r = jax.nn.sigmoid(ir + hr)
        z = jax.nn.sigmoid(iz + hz)
        n = jnp.tanh(i_n + r * h_n)
        h_new = (1.0 - z) * n + z * h
        pred = h_new @ W_fc + b_fc
        logp = jax.nn.log_softmax(pred, axis=1)
        # per-shard SUM of -logp at target (mean over full batch applied later)
        loss_t = -jnp.sum(jnp.take_along_axis(logp, tgt_t[:, None], axis=1))
        return (h_new, tgt_t), (pred, h_new, loss_t)

    h0 = jnp.zeros((efm.shape[0], HID), efm.dtype)
    tok0 = jnp.full((efm.shape[0],), START, tgt.dtype)
    (_, _), (preds, hs, losses) = jax.lax.scan(step, (h0, tok0), tgt.T)
    return preds, hs, losses  # [T,bs,V], [T,bs,H], [T]


_pm = None


def _get_pmapped():
    global _pm
    if _pm is None:
        _pm = jax.pmap(
            _decode_shard,
            in_axes=(0, 0) + (None,) * 12,
            axis_name='dp',
        )
    return _pm


def kernel(encoded_features_map, structural_target, embedding, We, be, Wh, bh,
           v_att, W_ih, W_hh, b_ih, b_hh, W_fc, b_fc):
    efm = np.asarray(encoded_features_map, np.float32).reshape(NCORES, BS, P, ENC)
    tgt_np = np.asarray(structural_target)
    tgt = tgt_np.reshape(NCORES, BS, T)
    params = [np.asarray(a, np.float32) for a in
              (embedding, We, be, Wh, bh, v_att, W_ih, W_hh, b_ih, b_hh, W_fc, b_fc)]
    pm = _get_pmapped()
    preds_s, hs_s, losses_s = pm(efm, tgt, *params)
    preds_s = np.asarray(preds_s)   # [8, T, bs, V]
    hs_s = np.asarray(hs_s)         # [8, T, bs, H]
    losses_s = np.asarray(losses_s)  # [8, T]
    preds = np.concatenate([preds_s[i] for i in range(NCORES)], axis=1)  # [T,B,V]
    hs = np.concatenate([hs_s[i] for i in range(NCORES)], axis=1)        # [T,B,H]
    loss = np.float32(losses_s.sum() / B)
    storage = hs[:, None, :, :]  # [T,1,B,H]
    return preds, loss, storage
